# revision 4
# baseline (speedup 1.0000x reference)
"""Trainium2 Bass kernel for nn_GatedCNNLayer.

Reference (X: (16, 4096, 1024) f32, G: (1024, 2), Gb: (2,)):
    lefts  = X[:, 0:L-2:2]; mids = X[:, 1:L-1:2]; rights = X[:, 2:L:2]
    gates  = softmax(mids @ G + Gb)                # (B, P, 2), P = 2047
    out    = lefts * gates[..., 0:1] + rights * gates[..., 1:2]

2-way softmax == sigmoid: g0 = sigmoid(mids @ w + bias), g1 = 1 - g0,
with w = G[:,0]-G[:,1], bias = Gb[0]-Gb[1] (precomputed on host).

Sharding: data-parallel over batch, 2 batches per core on 8 cores.

Layout: one output position per SBUF partition, D=1024 on the free dim.
Each chunk of 127 outputs loads 256 consecutive rows of X[b] as ONE
contiguous 1MB DMA into C[128, 2048] (partition p = rows 2p|2p+1 =
even|odd). lefts = C[0:127, 0:D], mids = C[0:127, D:2D], rights =
C[1:128, 0:D] realigned to partitions 0..126 by an SBUF->SBUF DMA
(engine ops cannot take partition-offset operands; DMA can).

Raw bass (no TileContext: this walrus build allows at most one attached
sync-wait per instruction, which Tile's scheduler violates), explicit
semaphores, 3-deep double buffering:
  sync   : C loads (HWDGE SP ring)
  scalar : R shift DMA (HWDGE ACT ring); sigmoid; g1; A = lefts*g0
  gpsimd : premul scr = mids*w ; output stores (SWDGE)
  vector : dot = reduce(scr); B = R*g1; out = A + B
Per-core HBM traffic ~33MB read + 17MB write -> ~140us roofline @358GB/s.
"""

import sys

sys.path.insert(0, "/opt/trn_rl_repo")

from contextlib import ExitStack

import numpy as np
from concourse import bass, mybir
from concourse.bass_utils import run_bass_kernel_spmd

f32 = mybir.dt.float32
FN = mybir.ActivationFunctionType
OP = mybir.AluOpType

B, L, D = 16, 4096, 1024
NCORES = 8
BPC = B // NCORES          # batches per core
P = L // 2 - 1             # outputs per batch = 2047
CHUNK = 127                # outputs per full chunk
NB = 3                     # buffer slots (C/R/scr/A/Bt/out)
NBS = 4                    # buffer slots for per-partition scalars

_cached = {}


def _chunks():
    out = []
    for b in range(BPC):
        p0 = 0
        while p0 < P:
            n = min(CHUNK, P - p0)
            out.append((b, p0, n))
            p0 += n
    return out


def build_nc():
    nc = bass.Bass()
    X = nc.dram_tensor("X", [BPC, L, D], f32, kind="ExternalInput")
    WB = nc.dram_tensor("WB", [128, D], f32, kind="ExternalInput")
    BBIAS = nc.dram_tensor("BBIAS", [128, 1], f32, kind="ExternalInput")
    OUT = nc.dram_tensor("OUT", [BPC, P, D], f32, kind="ExternalOutput")

    chunks = _chunks()
    NCH = len(chunks)

    with ExitStack() as ctx:
        block = ctx.enter_context(nc.Block())
        sem_c = ctx.enter_context(nc.semaphore("sem_const"))
        # DMA completion sems are PER BUFFER SLOT: a DMA's 16 per-engine
        # increments interleave with other in-flight DMAs on the same ring,
        # so a shared cumulative sem can hit 16*(j+1) while chunk j's DMA
        # is still partially outstanding. One sem per slot removes aliasing
        # between concurrently-running DMAs (slot reuse is already gated).
        sem_l = [ctx.enter_context(nc.semaphore(f"sem_load{k}"))
                 for k in range(NB)]
        sem_sh = [ctx.enter_context(nc.semaphore(f"sem_shift{k}"))
                  for k in range(NB)]
        sem_st = [ctx.enter_context(nc.semaphore(f"sem_store{k}"))
                  for k in range(NB)]
        sem_pm = ctx.enter_context(nc.semaphore("sem_premul"))
        sem_d1 = ctx.enter_context(nc.semaphore("sem_reduce"))
        sem_ac = ctx.enter_context(nc.semaphore("sem_scalar"))
        sem_d2 = ctx.enter_context(nc.semaphore("sem_blend"))

        wb = ctx.enter_context(nc.sbuf_tensor("wb", [128, D], f32))
        bb = ctx.enter_context(nc.sbuf_tensor("bb", [128, 1], f32))
        Cs = [ctx.enter_context(nc.sbuf_tensor(f"C{k}", [128, 2 * D], f32))
              for k in range(NB)]
        Rs = [ctx.enter_context(nc.sbuf_tensor(f"R{k}", [128, D], f32))
              for k in range(NB)]
        SCs = [ctx.enter_context(nc.sbuf_tensor(f"SC{k}", [128, D], f32))
               for k in range(NB)]
        As = [ctx.enter_context(nc.sbuf_tensor(f"A{k}", [128, D], f32))
              for k in range(NB)]
        Bts = [ctx.enter_context(nc.sbuf_tensor(f"Bt{k}", [128, D], f32))
               for k in range(NB)]
        Os = [ctx.enter_context(nc.sbuf_tensor(f"O{k}", [128, D], f32))
              for k in range(NB)]
        dots = [ctx.enter_context(nc.sbuf_tensor(f"dot{k}", [128, 1], f32))
                for k in range(NBS)]
        g0s = [ctx.enter_context(nc.sbuf_tensor(f"g0{k}", [128, 1], f32))
               for k in range(NBS)]
        g1s = [ctx.enter_context(nc.sbuf_tensor(f"g1{k}", [128, 1], f32))
               for k in range(NBS)]

        def gen(j):
            # generation of chunk j on its slot: wait value 16*gen(j) means
            # "slot sem has seen chunk j's DMA complete"
            return 16 * (j // NB + 1)

        @block.sync
        def _(sync):
            sync.dma_start(out=wb[:], in_=WB[:]).then_inc(sem_c, 16)
            sync.dma_start(out=bb[:], in_=BBIAS[:]).then_inc(sem_c, 16)
            for j, (b, p0, n) in enumerate(chunks):
                k = j % NB
                if j >= NB:
                    v = j - NB + 1
                    # C slot readers of chunk j-NB done:
                    sync.wait_ge(sem_pm, v)            # gpsimd premul (mids)
                    sync.wait_ge(sem_ac, v)            # scalar A (lefts)
                    sync.wait_ge(sem_sh[k], gen(j - NB))  # shift dma (evens)
                Ct = Cs[k]
                src = X[b, 2 * p0 : 2 * p0 + 2 * (n + 1), :].rearrange(
                    "(p t) d -> p (t d)", t=2
                )
                sync.dma_start(out=Ct[0 : n + 1, :], in_=src).then_inc(
                    sem_l[k], 16
                )
            for k in range(NB):                        # all stores landed
                nst = len([j for j in range(NCH) if j % NB == k])
                sync.wait_ge(sem_st[k], 16 * nst)

        @block.scalar
        def _(scalar):
            scalar.wait_ge(sem_c, 32)
            for j, (b, p0, n) in enumerate(chunks):
                k = j % NB
                Ct, Rt = Cs[k], Rs[k]
                dot, g0, g1 = dots[j % NBS], g0s[j % NBS], g1s[j % NBS]
                A = As[k]
                scalar.wait_ge(sem_l[k], gen(j))       # C_j loaded
                if j >= NB:
                    # R and A slot reuse: vector blend of chunk j-NB done
                    scalar.wait_ge(sem_d2, j - NB + 1)
                scalar.dma_start(
                    out=Rt[0:n, :], in_=Ct[1 : n + 1, 0:D]
                ).then_inc(sem_sh[k], 16)
                scalar.wait_ge(sem_d1, j + 1)          # dot_j ready
                scalar.activation(g0[0:n, :], dot[0:n, :], FN.Sigmoid,
                                  bias=bb[0:n, :], scale=1.0)
                scalar.activation(g1[0:n, :], g0[0:n, :], FN.Copy,
                                  bias=1.0, scale=-1.0)
                scalar.activation(A[0:n, :], Ct[0:n, 0:D], FN.Copy,
                                  bias=0.0, scale=g0[0:n, :]).then_inc(
                    sem_ac, 1
                )

        @block.gpsimd
        def _(gpsimd):
            gpsimd.wait_ge(sem_c, 32)
            for j, (b, p0, n) in enumerate(chunks):
                k = j % NB
                Ct, SCt = Cs[k], SCs[k]
                gpsimd.wait_ge(sem_l[k], gen(j))
                if j >= NB:
                    gpsimd.wait_ge(sem_d1, j - NB + 1)  # scr slot read
                gpsimd.tensor_mul(
                    SCt[0:n, :], Ct[0:n, D : 2 * D], wb[0:n, :]
                ).then_inc(sem_pm, 1)
                if j >= 1:
                    bp, pp, npp = chunks[j - 1]
                    gpsimd.wait_ge(sem_d2, j)           # out_{j-1} ready
                    gpsimd.dma_start(
                        out=OUT[bp, pp : pp + npp, :],
                        in_=Os[(j - 1) % NB][0:npp, :],
                    ).then_inc(sem_st[(j - 1) % NB], 16)
            bp, pp, npp = chunks[NCH - 1]
            gpsimd.wait_ge(sem_d2, NCH)
            gpsimd.dma_start(
                out=OUT[bp, pp : pp + npp, :],
                in_=Os[(NCH - 1) % NB][0:npp, :],
            ).then_inc(sem_st[(NCH - 1) % NB], 16)

        @block.vector
        def _(vector):
            for j, (b, p0, n) in enumerate(chunks):
                k = j % NB
                SCt, Rt = SCs[k], Rs[k]
                A, Bt, O = As[k], Bts[k], Os[k]
                dot, g1 = dots[j % NBS], g1s[j % NBS]
                vector.wait_ge(sem_pm, j + 1)
                if j >= NBS:
                    vector.wait_ge(sem_ac, j - NBS + 1)  # dot slot read
                vector.tensor_reduce(
                    dot[0:n, :], SCt[0:n, :],
                    axis=mybir.AxisListType.X, op=OP.add,
                ).then_inc(sem_d1, 1)
                vector.wait_ge(sem_ac, j + 1)            # g1_j, A_j ready
                vector.wait_ge(sem_sh[k], gen(j))        # R_j ready
                if j >= NB:
                    vector.wait_ge(sem_st[k], gen(j - NB))  # out slot free
                vector.tensor_scalar_mul(Bt[0:n, :], Rt[0:n, :], g1[0:n, :])
                vector.tensor_add(O[0:n, :], A[0:n, :], Bt[0:n, :]).then_inc(
                    sem_d2, 1
                )

    return nc


def _get_nc():
    if "nc" not in _cached:
        _cached["nc"] = build_nc()
    return _cached["nc"]


def kernel(X, G, Gb, trace=False, **trace_kwargs):
    X = np.ascontiguousarray(X, dtype=np.float32)
    G = np.asarray(G, dtype=np.float32)
    Gb = np.asarray(Gb, dtype=np.float32)
    w = G[:, 0] - G[:, 1]
    bias = np.float32(Gb[0] - Gb[1])
    WB = np.ascontiguousarray(np.broadcast_to(w, (128, D)))
    BB = np.full((128, 1), bias, dtype=np.float32)

    nc = _get_nc()
    in_maps = [
        {"X": X[i * BPC : (i + 1) * BPC], "WB": WB, "BBIAS": BB}
        for i in range(NCORES)
    ]
    res = run_bass_kernel_spmd(
        nc, in_maps, list(range(NCORES)), trace=trace, **trace_kwargs
    )
    out = np.concatenate([r["OUT"] for r in res.results], axis=0)
    if trace:
        return out, res
    return out


# revision 5
# speedup vs baseline: 1.1440x; 1.1440x over previous
"""Trainium2 Bass kernel for nn_GatedCNNLayer.

Reference (X: (16, 4096, 1024) f32, G: (1024, 2), Gb: (2,)):
    lefts  = X[:, 0:L-2:2]; mids = X[:, 1:L-1:2]; rights = X[:, 2:L:2]
    gates  = softmax(mids @ G + Gb)                # (B, P, 2), P = 2047
    out    = lefts * gates[..., 0:1] + rights * gates[..., 1:2]

2-way softmax == sigmoid: g0 = sigmoid(mids @ w + bias), g1 = 1 - g0,
with w = G[:,0]-G[:,1], bias = Gb[0]-Gb[1] (precomputed on host).

Sharding: data-parallel over batch, 2 batches per core on 8 cores.

Layout: one output position per SBUF partition, D=1024 on the free dim.
Each chunk of 127 outputs loads 256 consecutive rows of X[b] as ONE
contiguous 1MB DMA into C[128, 2048] (partition p = rows 2p|2p+1 =
even|odd). lefts = C[0:127, 0:D], mids = C[0:127, D:2D]. rights =
C[1:128, 0:D] realigned to partitions 0..126 by the (otherwise idle)
TensorE: a shift-permutation matmul into PSUM (engine ops cannot take
partition-offset operands, and an SBUF->SBUF shift DMA both costs
~3.7us of issue time and 1MB/chunk of DMA-engine bandwidth).

Raw bass (no TileContext: this walrus build allows at most one attached
sync-wait per instruction, which Tile's scheduler violates), explicit
semaphores, 3-deep buffering. Per-slot DMA semaphores: a DMA's 16
per-engine increments interleave with other in-flight DMAs on the same
ring, so one shared cumulative semaphore would fire early.

  sync   : C loads + output stores (HWDGE SP ring, spread over 16 SDMA)
  tensor : R_psum = ShiftPerm @ C_evens (2 fp32 matmuls, N=512 each)
  gpsimd : premul scr = mids * w
  vector : dot = reduce(scr); out = A + B
  scalar : g0 = sigmoid(dot+bias); g1 = 1-g0; A = lefts*g0;
           B = R_psum*g1 (PSUM -> SBUF activation, per-partition scale)

Per-core HBM traffic ~33MB read + 17MB write -> ~140us roofline @358GB/s.
"""

import sys

sys.path.insert(0, "/opt/trn_rl_repo")

from contextlib import ExitStack

import numpy as np
from concourse import bass, mybir
from concourse.bass_utils import run_bass_kernel_spmd

f32 = mybir.dt.float32
FN = mybir.ActivationFunctionType
OP = mybir.AluOpType

B, L, D = 16, 4096, 1024
NCORES = 8
BPC = B // NCORES          # batches per core
P = L // 2 - 1             # outputs per batch = 2047
CHUNK = 127                # outputs per full chunk
NB = 3                     # buffer slots (C/scr/A/Bt/out)
NBS = 4                    # buffer slots for per-partition scalars
NPB = 2                    # PSUM buffer slots

_cached = {}


def _chunks():
    out = []
    for b in range(BPC):
        p0 = 0
        while p0 < P:
            n = min(CHUNK, P - p0)
            out.append((b, p0, n))
            p0 += n
    return out


def build_nc():
    nc = bass.Bass()
    X = nc.dram_tensor("X", [BPC, L, D], f32, kind="ExternalInput")
    WB = nc.dram_tensor("WB", [128, D], f32, kind="ExternalInput")
    BBIAS = nc.dram_tensor("BBIAS", [128, 1], f32, kind="ExternalInput")
    SHIFT = nc.dram_tensor("SHIFT", [128, CHUNK], f32, kind="ExternalInput")
    OUT = nc.dram_tensor("OUT", [BPC, P, D], f32, kind="ExternalOutput")

    chunks = _chunks()
    NCH = len(chunks)

    with ExitStack() as ctx:
        block = ctx.enter_context(nc.Block())
        sem_c = ctx.enter_context(nc.semaphore("sem_const"))
        sem_l = [ctx.enter_context(nc.semaphore(f"sem_load{k}"))
                 for k in range(NB)]
        sem_st = [ctx.enter_context(nc.semaphore(f"sem_store{k}"))
                  for k in range(NB)]
        sem_pm = ctx.enter_context(nc.semaphore("sem_premul"))
        sem_d1 = ctx.enter_context(nc.semaphore("sem_reduce"))
        sem_ac = ctx.enter_context(nc.semaphore("sem_scalar"))
        sem_d2 = ctx.enter_context(nc.semaphore("sem_blend"))
        sem_mm = ctx.enter_context(nc.semaphore("sem_matmul"))

        wb = ctx.enter_context(nc.sbuf_tensor("wb", [128, D], f32))
        bb = ctx.enter_context(nc.sbuf_tensor("bb", [128, 1], f32))
        shm = ctx.enter_context(nc.sbuf_tensor("shm", [128, CHUNK], f32))
        Cs = [ctx.enter_context(nc.sbuf_tensor(f"C{k}", [128, 2 * D], f32))
              for k in range(NB)]
        SCs = [ctx.enter_context(nc.sbuf_tensor(f"SC{k}", [128, D], f32))
               for k in range(NB)]
        As = [ctx.enter_context(nc.sbuf_tensor(f"A{k}", [128, D], f32))
              for k in range(NB)]
        Bts = [ctx.enter_context(nc.sbuf_tensor(f"Bt{k}", [128, D], f32))
               for k in range(NB)]
        Os = [ctx.enter_context(nc.sbuf_tensor(f"O{k}", [128, D], f32))
              for k in range(NB)]
        dots = [ctx.enter_context(nc.sbuf_tensor(f"dot{k}", [128, 1], f32))
                for k in range(NBS)]
        g0s = [ctx.enter_context(nc.sbuf_tensor(f"g0{k}", [128, 1], f32))
               for k in range(NBS)]
        g1s = [ctx.enter_context(nc.sbuf_tensor(f"g1{k}", [128, 1], f32))
               for k in range(NBS)]
        PSs = [ctx.enter_context(nc.psum_tensor(f"PS{k}", [128, D], f32))
               for k in range(NPB)]

        def gen(j):
            # wait value meaning "slot sem has seen chunk j's DMA complete"
            return 16 * (j // NB + 1)

        @block.sync
        def _(sync):
            sync.dma_start(out=wb[:], in_=WB[:]).then_inc(sem_c, 16)
            sync.dma_start(out=bb[:], in_=BBIAS[:]).then_inc(sem_c, 16)
            sync.dma_start(out=shm[:], in_=SHIFT[:]).then_inc(sem_c, 16)
            for j, (b, p0, n) in enumerate(chunks):
                k = j % NB
                if j >= NB:
                    v = j - NB + 1
                    # C slot readers of chunk j-NB done:
                    sync.wait_ge(sem_pm, v)        # gpsimd premul (mids)
                    sync.wait_ge(sem_ac, v)        # scalar A (lefts)
                    sync.wait_ge(sem_mm, v)        # PE matmul (evens)
                Ct = Cs[k]
                src = X[b, 2 * p0 : 2 * p0 + 2 * (n + 1), :].rearrange(
                    "(p t) d -> p (t d)", t=2
                )
                sync.dma_start(out=Ct[0 : n + 1, :], in_=src).then_inc(
                    sem_l[k], 16
                )
                if j >= 1:
                    bp, pp, npp = chunks[j - 1]
                    sync.wait_ge(sem_d2, j)        # out_{j-1} ready
                    sync.dma_start(
                        out=OUT[bp, pp : pp + npp, :],
                        in_=Os[(j - 1) % NB][0:npp, :],
                    ).then_inc(sem_st[(j - 1) % NB], 16)
            bp, pp, npp = chunks[NCH - 1]
            sync.wait_ge(sem_d2, NCH)
            sync.dma_start(
                out=OUT[bp, pp : pp + npp, :],
                in_=Os[(NCH - 1) % NB][0:npp, :],
            ).then_inc(sem_st[(NCH - 1) % NB], 16)
            for k in range(NB):                    # all stores landed
                nst = len([j for j in range(NCH) if j % NB == k])
                sync.wait_ge(sem_st[k], 16 * nst)

        @block.tensor
        def _(tensor):
            tensor.wait_ge(sem_c, 48)
            for j, (b, p0, n) in enumerate(chunks):
                k = j % NB
                Ct, PS = Cs[k], PSs[j % NPB]
                tensor.wait_ge(sem_l[k], gen(j))   # C_j loaded
                if j >= NPB:
                    # PSUM slot reuse: scalar B-copy of chunk j-NPB done
                    tensor.wait_ge(sem_ac, j - NPB + 1)
                tensor.matmul(
                    PS[0:n, 0:512], shm[0 : n + 1, 0:n], Ct[0 : n + 1, 0:512],
                    start=True, stop=True,
                )
                tensor.matmul(
                    PS[0:n, 512:1024], shm[0 : n + 1, 0:n],
                    Ct[0 : n + 1, 512:1024],
                    start=True, stop=True,
                ).then_inc(sem_mm, 1)

        @block.gpsimd
        def _(gpsimd):
            gpsimd.wait_ge(sem_c, 48)
            for j, (b, p0, n) in enumerate(chunks):
                k = j % NB
                Ct, SCt = Cs[k], SCs[k]
                gpsimd.wait_ge(sem_l[k], gen(j))
                if j >= NB:
                    gpsimd.wait_ge(sem_d1, j - NB + 1)  # scr slot read
                gpsimd.tensor_mul(
                    SCt[0:n, :], Ct[0:n, D : 2 * D], wb[0:n, :]
                ).then_inc(sem_pm, 1)

        @block.vector
        def _(vector):
            for j, (b, p0, n) in enumerate(chunks):
                k = j % NB
                SCt = SCs[k]
                A, Bt, O = As[k], Bts[k], Os[k]
                dot = dots[j % NBS]
                vector.wait_ge(sem_pm, j + 1)
                if j >= NBS:
                    vector.wait_ge(sem_ac, j - NBS + 1)  # dot slot read
                vector.tensor_reduce(
                    dot[0:n, :], SCt[0:n, :],
                    axis=mybir.AxisListType.X, op=OP.add,
                ).then_inc(sem_d1, 1)
                vector.wait_ge(sem_ac, j + 1)            # A_j, B_j ready
                if j >= NB:
                    vector.wait_ge(sem_st[k], gen(j - NB))  # out slot free
                vector.tensor_add(O[0:n, :], A[0:n, :], Bt[0:n, :]).then_inc(
                    sem_d2, 1
                )

        @block.scalar
        def _(scalar):
            scalar.wait_ge(sem_c, 48)
            for j, (b, p0, n) in enumerate(chunks):
                k = j % NB
                Ct, PS = Cs[k], PSs[j % NPB]
                dot, g0, g1 = dots[j % NBS], g0s[j % NBS], g1s[j % NBS]
                A, Bt = As[k], Bts[k]
                scalar.wait_ge(sem_d1, j + 1)          # dot_j ready
                scalar.activation(g0[0:n, :], dot[0:n, :], FN.Sigmoid,
                                  bias=bb[0:n, :], scale=1.0)
                scalar.activation(g1[0:n, :], g0[0:n, :], FN.Copy,
                                  bias=1.0, scale=-1.0)
                if j >= NB:
                    # A/Bt slot reuse: vector blend of chunk j-NB done
                    scalar.wait_ge(sem_d2, j - NB + 1)
                scalar.activation(A[0:n, :], Ct[0:n, 0:D], FN.Copy,
                                  bias=0.0, scale=g0[0:n, :])
                scalar.wait_ge(sem_mm, j + 1)          # R_psum ready
                scalar.activation(Bt[0:n, :], PS[0:n, :], FN.Copy,
                                  bias=0.0, scale=g1[0:n, :]).then_inc(
                    sem_ac, 1
                )

    return nc


def _get_nc():
    if "nc" not in _cached:
        _cached["nc"] = build_nc()
    return _cached["nc"]


def kernel(X, G, Gb, trace=False, **trace_kwargs):
    X = np.ascontiguousarray(X, dtype=np.float32)
    G = np.asarray(G, dtype=np.float32)
    Gb = np.asarray(Gb, dtype=np.float32)
    w = G[:, 0] - G[:, 1]
    bias = np.float32(Gb[0] - Gb[1])
    WB = np.ascontiguousarray(np.broadcast_to(w, (128, D)))
    BB = np.full((128, 1), bias, dtype=np.float32)
    # shift permutation: out[m,:] = evens[m+1,:]  ->  S[k,m] = 1 iff k==m+1
    SH = np.zeros((128, CHUNK), dtype=np.float32)
    for m in range(CHUNK):
        SH[m + 1, m] = 1.0

    nc = _get_nc()
    in_maps = [
        {"X": X[i * BPC : (i + 1) * BPC], "WB": WB, "BBIAS": BB, "SHIFT": SH}
        for i in range(NCORES)
    ]
    res = run_bass_kernel_spmd(
        nc, in_maps, list(range(NCORES)), trace=trace, **trace_kwargs
    )
    out = np.concatenate([r["OUT"] for r in res.results], axis=0)
    if trace:
        return out, res
    return out


# revision 6
# speedup vs baseline: 2.6085x; 2.2802x over previous
"""Trainium2 Bass kernel for nn_GatedCNNLayer.

Reference (X: (16, 4096, 1024) f32, G: (1024, 2), Gb: (2,)):
    lefts  = X[:, 0:L-2:2]; mids = X[:, 1:L-1:2]; rights = X[:, 2:L:2]
    gates  = softmax(mids @ G + Gb)                # (B, P, 2), P = 2047
    out    = lefts * gates[..., 0:1] + rights * gates[..., 1:2]

2-way softmax == sigmoid: g0 = sigmoid(mids @ w + bias), g1 = 1 - g0,
with w = G[:,0]-G[:,1], bias = Gb[0]-Gb[1] (precomputed on host).

Sharding: data-parallel over batch, 2 batches per core on 8 cores.

Layout: one output position per SBUF partition, D=1024 on the free dim.
Each chunk of 127 outputs loads 256 consecutive rows of X[b] as ONE
contiguous 1MB DMA into C[128, 2048] (partition p = rows 2p|2p+1 =
even|odd). lefts = C[0:127, 0:D], mids = C[0:127, D:2D]. rights =
C[1:128, 0:D] realigned to partitions 0..126 by the (otherwise idle)
TensorE: a shift-permutation matmul into PSUM (engine ops cannot take
partition-offset operands, and an SBUF->SBUF shift DMA both costs
~3.7us of issue time and 1MB/chunk of DMA-engine bandwidth).

Raw bass (no TileContext: this walrus build allows at most one attached
sync-wait per instruction, which Tile's scheduler violates), explicit
semaphores, 3-deep buffering. Per-slot DMA semaphores: a DMA's 16
per-engine increments interleave with other in-flight DMAs on the same
ring, so one shared cumulative semaphore would fire early.

  sync   : C loads + output stores (HWDGE SP ring, spread over 16 SDMA)
  tensor : R_psum = ShiftPerm @ C_evens (2 fp32 matmuls, N=512 each)
  gpsimd : premul scr = mids * w
  vector : dot = reduce(scr); out = A + B
  scalar : g0 = sigmoid(dot+bias); g1 = 1-g0; A = lefts*g0;
           B = R_psum*g1 (PSUM -> SBUF activation, per-partition scale)

Per-core HBM traffic ~33MB read + 17MB write -> ~140us roofline @358GB/s.
"""

import sys

sys.path.insert(0, "/opt/trn_rl_repo")

from contextlib import ExitStack

import numpy as np
from concourse import bass, mybir
from concourse.bass_utils import run_bass_kernel_spmd

f32 = mybir.dt.float32
FN = mybir.ActivationFunctionType
OP = mybir.AluOpType

B, L, D = 16, 4096, 1024
NCORES = 8
BPC = B // NCORES          # batches per core
P = L // 2 - 1             # outputs per batch = 2047
CHUNK = 126                # outputs per chunk: stores of 126
                           # partitions spread across all 16 SDMA
                           # engines; 127 collapses onto one
NB = 3                     # buffer slots (C/scr/A/Bt/out)
NBS = 4                    # buffer slots for per-partition scalars
NPB = 2                    # PSUM buffer slots

_cached = {}


def _chunks():
    out = []
    for b in range(BPC):
        p0 = 0
        while p0 < P:
            n = min(CHUNK, P - p0)
            out.append((b, p0, n))
            p0 += n
    return out


def build_nc():
    nc = bass.Bass()
    X = nc.dram_tensor("X", [BPC, L, D], f32, kind="ExternalInput")
    WB = nc.dram_tensor("WB", [128, D], f32, kind="ExternalInput")
    BBIAS = nc.dram_tensor("BBIAS", [128, 1], f32, kind="ExternalInput")
    SHIFT = nc.dram_tensor("SHIFT", [128, CHUNK], f32, kind="ExternalInput")
    OUT = nc.dram_tensor("OUT", [BPC, P, D], f32, kind="ExternalOutput")

    chunks = _chunks()
    NCH = len(chunks)

    with ExitStack() as ctx:
        block = ctx.enter_context(nc.Block())
        sem_c = ctx.enter_context(nc.semaphore("sem_const"))
        sem_l = [ctx.enter_context(nc.semaphore(f"sem_load{k}"))
                 for k in range(NB)]
        sem_st = [ctx.enter_context(nc.semaphore(f"sem_store{k}"))
                  for k in range(NB)]
        sem_pm = ctx.enter_context(nc.semaphore("sem_premul"))
        sem_d1 = ctx.enter_context(nc.semaphore("sem_reduce"))
        sem_ac = ctx.enter_context(nc.semaphore("sem_scalar"))
        sem_d2 = ctx.enter_context(nc.semaphore("sem_blend"))
        sem_mm = ctx.enter_context(nc.semaphore("sem_matmul"))

        wb = ctx.enter_context(nc.sbuf_tensor("wb", [128, D], f32))
        bb = ctx.enter_context(nc.sbuf_tensor("bb", [128, 1], f32))
        shm = ctx.enter_context(nc.sbuf_tensor("shm", [128, CHUNK], f32))
        Cs = [ctx.enter_context(nc.sbuf_tensor(f"C{k}", [128, 2 * D], f32))
              for k in range(NB)]
        SCs = [ctx.enter_context(nc.sbuf_tensor(f"SC{k}", [128, D], f32))
               for k in range(NB)]
        As = [ctx.enter_context(nc.sbuf_tensor(f"A{k}", [128, D], f32))
              for k in range(NB)]
        Bts = [ctx.enter_context(nc.sbuf_tensor(f"Bt{k}", [128, D], f32))
               for k in range(NB)]
        Os = [ctx.enter_context(nc.sbuf_tensor(f"O{k}", [128, D], f32))
              for k in range(NB)]
        dots = [ctx.enter_context(nc.sbuf_tensor(f"dot{k}", [128, 1], f32))
                for k in range(NBS)]
        g0s = [ctx.enter_context(nc.sbuf_tensor(f"g0{k}", [128, 1], f32))
               for k in range(NBS)]
        g1s = [ctx.enter_context(nc.sbuf_tensor(f"g1{k}", [128, 1], f32))
               for k in range(NBS)]
        PSs = [ctx.enter_context(nc.psum_tensor(f"PS{k}", [128, D], f32))
               for k in range(NPB)]

        def gen(j):
            # wait value meaning "slot sem has seen chunk j's DMA complete"
            return 16 * (j // NB + 1)

        @block.sync
        def _(sync):
            sync.dma_start(out=wb[:], in_=WB[:]).then_inc(sem_c, 16)
            sync.dma_start(out=bb[:], in_=BBIAS[:]).then_inc(sem_c, 16)
            sync.dma_start(out=shm[:], in_=SHIFT[:]).then_inc(sem_c, 16)
            for j, (b, p0, n) in enumerate(chunks):
                k = j % NB
                if j >= NB:
                    v = j - NB + 1
                    # C slot readers of chunk j-NB done:
                    sync.wait_ge(sem_pm, v)        # gpsimd premul (mids)
                    sync.wait_ge(sem_ac, v)        # scalar A (lefts)
                    sync.wait_ge(sem_mm, v)        # PE matmul (evens)
                Ct = Cs[k]
                npl = min(128, (L - 2 * p0) // 2)   # load partitions (128
                # when possible: 127-partition DMAs also skew onto one engine)
                src = X[b, 2 * p0 : 2 * p0 + 2 * npl, :].rearrange(
                    "(p t) d -> p (t d)", t=2
                )
                sync.dma_start(out=Ct[0:npl, :], in_=src).then_inc(
                    sem_l[k], 16
                )
                if j >= 1:
                    bp, pp, npp = chunks[j - 1]
                    sync.wait_ge(sem_d2, j)        # out_{j-1} ready
                    sync.dma_start(
                        out=OUT[bp, pp : pp + npp, :],
                        in_=Os[(j - 1) % NB][0:npp, :],
                    ).then_inc(sem_st[(j - 1) % NB], 16)
            bp, pp, npp = chunks[NCH - 1]
            sync.wait_ge(sem_d2, NCH)
            sync.dma_start(
                out=OUT[bp, pp : pp + npp, :],
                in_=Os[(NCH - 1) % NB][0:npp, :],
            ).then_inc(sem_st[(NCH - 1) % NB], 16)
            for k in range(NB):                    # all stores landed
                nst = len([j for j in range(NCH) if j % NB == k])
                sync.wait_ge(sem_st[k], 16 * nst)

        @block.tensor
        def _(tensor):
            tensor.wait_ge(sem_c, 48)
            for j, (b, p0, n) in enumerate(chunks):
                k = j % NB
                Ct, PS = Cs[k], PSs[j % NPB]
                tensor.wait_ge(sem_l[k], gen(j))   # C_j loaded
                if j >= NPB:
                    # PSUM slot reuse: scalar B-copy of chunk j-NPB done
                    tensor.wait_ge(sem_ac, j - NPB + 1)
                tensor.matmul(
                    PS[0:n, 0:512], shm[0 : n + 1, 0:n], Ct[0 : n + 1, 0:512],
                    start=True, stop=True,
                )
                tensor.matmul(
                    PS[0:n, 512:1024], shm[0 : n + 1, 0:n],
                    Ct[0 : n + 1, 512:1024],
                    start=True, stop=True,
                ).then_inc(sem_mm, 1)

        @block.gpsimd
        def _(gpsimd):
            gpsimd.wait_ge(sem_c, 48)
            for j, (b, p0, n) in enumerate(chunks):
                k = j % NB
                Ct, SCt = Cs[k], SCs[k]
                gpsimd.wait_ge(sem_l[k], gen(j))
                if j >= NB:
                    gpsimd.wait_ge(sem_d1, j - NB + 1)  # scr slot read
                gpsimd.tensor_mul(
                    SCt[0:n, :], Ct[0:n, D : 2 * D], wb[0:n, :]
                ).then_inc(sem_pm, 1)

        @block.vector
        def _(vector):
            for j, (b, p0, n) in enumerate(chunks):
                k = j % NB
                SCt = SCs[k]
                A, Bt, O = As[k], Bts[k], Os[k]
                dot = dots[j % NBS]
                vector.wait_ge(sem_pm, j + 1)
                if j >= NBS:
                    vector.wait_ge(sem_ac, j - NBS + 1)  # dot slot read
                vector.tensor_reduce(
                    dot[0:n, :], SCt[0:n, :],
                    axis=mybir.AxisListType.X, op=OP.add,
                ).then_inc(sem_d1, 1)
                vector.wait_ge(sem_ac, j + 1)            # A_j, B_j ready
                if j >= NB:
                    vector.wait_ge(sem_st[k], gen(j - NB))  # out slot free
                vector.tensor_add(O[0:n, :], A[0:n, :], Bt[0:n, :]).then_inc(
                    sem_d2, 1
                )

        @block.scalar
        def _(scalar):
            scalar.wait_ge(sem_c, 48)
            for j, (b, p0, n) in enumerate(chunks):
                k = j % NB
                Ct, PS = Cs[k], PSs[j % NPB]
                dot, g0, g1 = dots[j % NBS], g0s[j % NBS], g1s[j % NBS]
                A, Bt = As[k], Bts[k]
                scalar.wait_ge(sem_d1, j + 1)          # dot_j ready
                scalar.activation(g0[0:n, :], dot[0:n, :], FN.Sigmoid,
                                  bias=bb[0:n, :], scale=1.0)
                scalar.activation(g1[0:n, :], g0[0:n, :], FN.Copy,
                                  bias=1.0, scale=-1.0)
                if j >= NB:
                    # A/Bt slot reuse: vector blend of chunk j-NB done
                    scalar.wait_ge(sem_d2, j - NB + 1)
                scalar.activation(A[0:n, :], Ct[0:n, 0:D], FN.Copy,
                                  bias=0.0, scale=g0[0:n, :])
                scalar.wait_ge(sem_mm, j + 1)          # R_psum ready
                scalar.activation(Bt[0:n, :], PS[0:n, :], FN.Copy,
                                  bias=0.0, scale=g1[0:n, :]).then_inc(
                    sem_ac, 1
                )

    return nc


def _get_nc():
    if "nc" not in _cached:
        _cached["nc"] = build_nc()
    return _cached["nc"]


def kernel(X, G, Gb, trace=False, **trace_kwargs):
    X = np.ascontiguousarray(X, dtype=np.float32)
    G = np.asarray(G, dtype=np.float32)
    Gb = np.asarray(Gb, dtype=np.float32)
    w = G[:, 0] - G[:, 1]
    bias = np.float32(Gb[0] - Gb[1])
    WB = np.ascontiguousarray(np.broadcast_to(w, (128, D)))
    BB = np.full((128, 1), bias, dtype=np.float32)
    # shift permutation: out[m,:] = evens[m+1,:]  ->  S[k,m] = 1 iff k==m+1
    SH = np.zeros((128, CHUNK), dtype=np.float32)
    for m in range(CHUNK):
        SH[m + 1, m] = 1.0

    nc = _get_nc()
    in_maps = [
        {"X": X[i * BPC : (i + 1) * BPC], "WB": WB, "BBIAS": BB, "SHIFT": SH}
        for i in range(NCORES)
    ]
    res = run_bass_kernel_spmd(
        nc, in_maps, list(range(NCORES)), trace=trace, **trace_kwargs
    )
    out = np.concatenate([r["OUT"] for r in res.results], axis=0)
    if trace:
        return out, res
    return out
